# revision 11
# baseline (speedup 1.0000x reference)
"""AugmentedLSTM Trainium2 kernel: 8 NeuronCores, 1 chip.

Decomposition: 2 batch-half groups x 4 gate-slice cores.
  core r (0..7): g = r & 3 owns H-slice [128g, 128g+128); bh = r >> 2 owns
  batch rows [32bh, 32bh+32).
Phase 1 (proj): pi = x @ w_in.T + b_in + b_state (per-core slice, 768 cols
  = 4 strips x [i f o hw gg p5] with hw negated), stored to HBM.
Phase 2 (recurrence): per step t, each core computes its 128 H-columns of
  the LSTM update for its 32-batch rows, then broadcasts the 32x32-block-
  transposed h-chunk [128,32] (bf16) to all 4 group members via SWDGE
  remote DMA (slot = sender's g, register offset); the 4 slots form h^T
  [512, 32] = stationary operand of the next step's gate matmuls.

Self-contained: hardcodes shapes B=64 T=512 D=512 H=512.
"""

import os
import sys
import types

import numpy as np
import ml_dtypes

BF16 = ml_dtypes.bfloat16
B, D, H = 64, 512, 512
FULL_T = 512
W6 = [0, 1, 3, 4, 2, 5]  # strip gate order [i f o hw gg p5] -> w_in block
W5 = [0, 1, 3, 4, 2]  # [i f o hw gg] -> w_state block


# ---------------------------------------------------------------- host prep
def prep_core(r, x, lengths, w_in, b_in, w_state, b_state, T):
    g, bh = r & 3, r >> 2
    xs = x[32 * bh : 32 * bh + 32, :T]  # [32, T, D]
    xt = np.ascontiguousarray(xs.transpose(2, 1, 0).reshape(D, T * 32)).astype(
        np.float32
    )

    wpt = np.empty((D, 768), dtype=np.float32)
    bias = np.empty((1, 768), dtype=np.float32)
    wrt = np.empty((H, 640), dtype=np.float32)
    for s in range(4):
        for q in range(6):
            sign = -1.0 if q == 3 else 1.0
            rows = W6[q] * H + 128 * g + 32 * s + np.arange(32)
            wpt[:, 192 * s + 32 * q : 192 * s + 32 * q + 32] = sign * w_in[rows, :].T
            bb = b_in[rows].copy()
            if q < 5:
                srows = W5[q] * H + 128 * g + 32 * s + np.arange(32)
                bb = bb + b_state[srows]
            bias[0, 192 * s + 32 * q : 192 * s + 32 * q + 32] = sign * bb
        for q in range(5):
            sign = -1.0 if q == 3 else 1.0
            srows = W5[q] * H + 128 * g + 32 * s + np.arange(32)
            wrt[:, 160 * s + 32 * q : 160 * s + 32 * q + 32] = sign * w_state[srows, :].T

    mask = np.zeros((128, T), dtype=np.float32)
    tt = np.arange(T)
    for s in range(4):
        for b_ in range(32):
            mask[32 * s + b_, :] = (tt < lengths[32 * bh + b_]).astype(np.float32)

    return {
        "xt": xt,
        "wpt": wpt,
        "biasrow": bias,
        "wrt": wrt.astype(BF16),
        "maskT": mask,
        "ident": np.tile(np.eye(32, dtype=np.float32), (4, 1)),
        "ones1": np.ones((1, 128), dtype=np.float32),
        "soff": np.array([[g]], dtype=np.uint32),
    }


# ---------------------------------------------------------------- bass build
def build_nc(T):
    import concourse.bass as bass
    import concourse.mybir as mybir
    from concourse import bacc

    f32 = mybir.dt.float32
    bf16 = mybir.dt.bfloat16
    u32 = mybir.dt.uint32
    AF = mybir.ActivationFunctionType

    M = T * 32 // 128  # proj M-tiles (= T/4)
    NT = T // 4  # pi tiles
    NO = T // 16  # out DMAs

    nc = bacc.Bacc("TRN2", num_devices=8)

    xt_d = nc.dram_tensor("xt", [D, T * 32], f32, kind="ExternalInput")
    wpt_d = nc.dram_tensor("wpt", [D, 768], f32, kind="ExternalInput")
    bias_d = nc.dram_tensor("biasrow", [1, 768], f32, kind="ExternalInput")
    wrt_d = nc.dram_tensor("wrt", [H, 640], bf16, kind="ExternalInput")
    mask_d = nc.dram_tensor("maskT", [128, T], f32, kind="ExternalInput")
    ident_d = nc.dram_tensor("ident", [128, 32], f32, kind="ExternalInput")
    ones_d = nc.dram_tensor("ones1", [1, 128], f32, kind="ExternalInput")
    soff_d = nc.dram_tensor("soff", [1, 1], u32, kind="ExternalInput")
    pihbm = nc.dram_tensor("pihbm", [NT, 128, 768], f32)
    outd = nc.dram_tensor("outd", [128, T * 32], bf16, kind="ExternalOutput")

    ctx = [
        nc.sbuf_tensor("xt_sb", [128, 8 * 128], f32),
        nc.sbuf_tensor("wpt_sb", [128, 4 * 768], f32),
        nc.sbuf_tensor("bias_sb", [1, 768], f32),
        nc.sbuf_tensor("ones_sb", [1, 128], f32),
        nc.sbuf_tensor("pistage", [128, 2 * 768], f32),
        nc.sbuf_tensor("wrt_sb", [128, 4 * 640], bf16),
        nc.sbuf_tensor("pi_sb", [128, 2 * 768], f32),
        nc.sbuf_tensor("ident_sb", [128, 32], f32),
        nc.sbuf_tensor("mask_sb", [128, T], f32),
        nc.sbuf_tensor("hslot", [128, 2 * 4 * 32], bf16),
        nc.sbuf_tensor("scm", [128, 2 * 32], bf16),
        nc.sbuf_tensor("gate_sb", [128, 160], f32),
        nc.sbuf_tensor("p5_sb", [128, 32], f32),
        nc.sbuf_tensor("c_sb", [128, 2 * 32], f32),
        nc.sbuf_tensor("chain", [128, 8 * 32], f32),
        nc.sbuf_tensor("outstage", [128, 2 * 512], bf16),
        nc.psum_tensor("pp0", [128, 768], f32),
        nc.psum_tensor("pp1", [128, 768], f32),
        nc.psum_tensor("pg0", [128, 192], f32),
        nc.psum_tensor("pg1", [128, 192], f32),
    ]
    sems = [
        nc.semaphore("s_wdma"),
        nc.semaphore("s_xd0"),
        nc.semaphore("s_xd1"),
        nc.semaphore("s_pe1"),
        nc.semaphore("s_cp1"),
        nc.semaphore("s_pio0"),
        nc.semaphore("s_pio1"),
        nc.semaphore("s_pid0"),
        nc.semaphore("s_pid1"),
        nc.semaphore("s_seed"),
        nc.semaphore("s_mm"),
        nc.semaphore("s_act"),
        nc.semaphore("s_cn"),
        nc.semaphore("s_th"),
        nc.semaphore("s_psumfree"),
        nc.semaphore("s_dve"),
        nc.semaphore("s_od0"),
        nc.semaphore("s_od1"),
        nc.semaphore("s_prep"),
        nc.semaphore("s_ls0"),
        nc.semaphore("s_ls1"),
        nc.semaphore("s_rs0"),
        nc.semaphore("s_rs1"),
        nc.semaphore("s_init"),
        nc.semaphore("s_vch"),
        nc.semaphore("s_sch"),
    ]
    import contextlib

    stack = contextlib.ExitStack()
    (
        xt_sb, wpt_sb, bias_sb, ones_sb, pistage, wrt_sb, pi_sb, ident_sb,
        mask_sb, hslot, scm, gate_sb, p5_sb, c_sb, chain, outstage,
        pp0, pp1, pg0, pg1,
    ) = [stack.enter_context(c) for c in ctx]
    (
        s_wdma, s_xd0, s_xd1, s_pe1, s_cp1, s_pio0, s_pio1, s_pid0, s_pid1,
        s_seed, s_mm, s_act, s_cn, s_th, s_psumfree, s_dve, s_od0, s_od1,
        s_prep, s_ls0, s_ls1, s_rs0, s_rs1, s_init, s_vch, s_sch,
    ) = [stack.enter_context(s) for s in sems]
    s_xd = [s_xd0, s_xd1]
    s_pio = [s_pio0, s_pio1]
    s_pid = [s_pid0, s_pid1]
    s_od = [s_od0, s_od1]
    s_ls = [s_ls0, s_ls1]
    s_rs = [s_rs0, s_rs1]
    pp = [pp0, pp1]
    pg = [pg0, pg1]

    with stack, nc.Block() as block:

        # chain-index helpers: every DVE op incs s_vch once; every ACT op
        # incs s_sch once. Cross-engine waits use computed indices.
        VINIT = 3   # DVE init memsets
        VPS = 10    # DVE ops per step
        APS = 4     # ACT ops per step

        def vidx(t, k):
            return VINIT + VPS * t + k

        def sidx(t, k):
            return M + APS * t + k

        # ---------------- sync engine: all HWDGE DMAs -------------------
        @block.sync
        def _(sy):
            ninit = 0
            for k in range(4):
                sy.dma_start(
                    out=wpt_sb[:, k * 768 : (k + 1) * 768],
                    in_=wpt_d[128 * k : 128 * (k + 1), :],
                ).then_inc(s_wdma, 16)
                ninit += 1
            for k in range(4):
                sy.dma_start(
                    out=wrt_sb[:, k * 640 : (k + 1) * 640],
                    in_=wrt_d[128 * k : 128 * (k + 1), :],
                ).then_inc(s_wdma, 16)
                ninit += 1
            for src, dst in (
                (mask_d, mask_sb),
                (ident_d, ident_sb),
                (ones_d, ones_sb),
                (bias_d, bias_sb),
            ):
                sy.dma_start(out=dst[:, :], in_=src[:, :]).then_inc(s_wdma, 16)
                ninit += 1
            assert ninit == 12

            # phase 1: xt tile loads + pi writeback
            for m in range(M):
                if m >= 2:
                    sy.wait_ge(s_pe1, m - 1)
                for k in range(4):
                    sy.dma_start(
                        out=xt_sb[:, ((m % 2) * 4 + k) * 128 : ((m % 2) * 4 + k + 1) * 128],
                        in_=xt_d[128 * k : 128 * (k + 1), 128 * m : 128 * (m + 1)],
                    ).then_inc(s_xd[m % 2], 16)
                if m >= 1:
                    j = m - 1
                    sy.wait_ge(s_sch, m)
                    sy.dma_start(
                        out=pihbm[j, :, :],
                        in_=pistage[:, (j % 2) * 768 : (j % 2) * 768 + 768],
                    ).then_inc(s_pio[j % 2], 16)
            j = M - 1
            sy.wait_ge(s_sch, M)
            sy.dma_start(
                out=pihbm[j, :, :],
                in_=pistage[:, (j % 2) * 768 : (j % 2) * 768 + 768],
            ).then_inc(s_pio[j % 2], 16)

            # phase 2: pi tile loads + out stores (in chronological order)
            def pi_load(m):
                if m >= 2:
                    sy.wait_ge(s_seed, max(0, 4 * m - 4))
                sy.dma_start(
                    out=pi_sb[:, (m % 2) * 768 : (m % 2) * 768 + 768],
                    in_=pihbm[m, :, :],
                ).then_inc(s_pid[m % 2], 16)

            pi_load(0)
            pi_load(1)
            nxt = 2
            for t in range(T):
                if t % 4 == 0 and nxt < NT:
                    pi_load(nxt)
                    nxt += 1
                if t % 16 == 15:
                    k = t // 16
                    sy.wait_ge(s_vch, 3 + 10 * t + 8)
                    sy.dma_start(
                        out=outd[:, k * 512 : (k + 1) * 512],
                        in_=outstage[:, (k % 2) * 512 : (k % 2) * 512 + 512],
                    ).then_inc(s_od[k % 2], 16)

        # ---------------- tensor engine --------------------------------
        @block.tensor
        def _(te):
            te.wait_ge(s_wdma, 16 * 12)
            # phase 1
            for m in range(M):
                te.wait_ge(s_xd[m % 2], 64 * (m // 2 + 1))
                if m >= 2:
                    te.wait_ge(s_sch, m - 1)
                last = None
                for k in range(4):
                    lhsT = xt_sb[:, ((m % 2) * 4 + k) * 128 : ((m % 2) * 4 + k + 1) * 128]
                    te.matmul(
                        pp[m % 2][:, 0:512],
                        lhsT,
                        wpt_sb[:, k * 768 : k * 768 + 512],
                        start=(k == 0),
                        stop=False,
                        skip_group_check=True,
                    )
                    last = te.matmul(
                        pp[m % 2][:, 512:768],
                        lhsT,
                        wpt_sb[:, k * 768 + 512 : k * 768 + 768],
                        start=(k == 0),
                        stop=False,
                        skip_group_check=True,
                    )
                te.matmul(
                    pp[m % 2][:, 0:512],
                    ones_sb[:, :],
                    bias_sb[:, 0:512],
                    start=False,
                    stop=True,
                    skip_group_check=True,
                )
                te.matmul(
                    pp[m % 2][:, 512:768],
                    ones_sb[:, :],
                    bias_sb[:, 512:768],
                    start=False,
                    stop=True,
                    skip_group_check=True,
                ).then_inc(s_pe1, 1)

            # phase 2
            te.wait_ge(s_vch, VINIT)
            for t in range(T):
                par = t % 2
                m, tt = divmod(t, 4)
                if tt == 0:
                    te.wait_ge(s_pid[m % 2], 16 * (m // 2 + 1))
                if t >= 2:
                    te.wait_ge(s_sch, sidx(t - 2, 3))
                # seed: pg[par][32s:32s+32, 0:192] = I32^T @ pi rows
                lastseed = None
                for s in range(4):
                    off = (m % 2) * 768 + 192 * s
                    lastseed = te.matmul(
                        pg[par][32 * s : 32 * s + 32, 0:192],
                        ident_sb[32 * tt : 32 * tt + 32, :],
                        pi_sb[32 * tt : 32 * tt + 32, off : off + 192],
                        start=True,
                        stop=False,
                        tile_position=(32 * tt, 32 * s),
                        skip_group_check=True,
                    )
                lastseed.then_inc(s_seed, 1)
                if t >= 1:
                    te.wait_ge(s_rs[t % 2], 8 * ((t - 1) // 2 + 1))
                lastmm = None
                for k in range(4):
                    for s in range(4):
                        lastmm = te.matmul(
                            pg[par][32 * s : 32 * s + 32, 0:160],
                            hslot[:, par * 128 + 32 * k : par * 128 + 32 * k + 32],
                            wrt_sb[:, k * 640 + 160 * s : k * 640 + 160 * s + 160],
                            start=False,
                            stop=(k == 3 and s == 3),
                            tile_position=(0, 32 * s),
                            skip_group_check=True,
                        )
                lastmm.then_inc(s_mm, 1)

        # ---------------- scalar engine (ACT) ---------------------------
        @block.scalar
        def _(sc):
            n = [0]

            def schop(instr):
                n[0] += 1
                instr.then_inc(s_sch, 1)
                return instr

            # phase 1 copies psum->pistage (serialized via s_sch)
            for m in range(M):
                sc.wait_ge(s_pe1, m + 1)
                if m >= 2:
                    sc.wait_ge(s_pio[m % 2], 16 * ((m - 2) // 2 + 1))
                if n[0]:
                    sc.wait_ge(s_sch, n[0])
                schop(
                    sc.copy(
                        pistage[:, (m % 2) * 768 : (m % 2) * 768 + 768],
                        pp[m % 2][:, :],
                    )
                )
            # phase 2
            for t in range(T):
                par = t % 2
                sc.wait_ge(s_mm, t + 1)
                if t >= 1:
                    sc.wait_ge(s_vch, vidx(t - 1, 6))  # DVE done reading gate_sb/p5
                sc.wait_ge(s_sch, n[0])
                schop(sc.activation(gate_sb[:, 0:128], pg[par][:, 0:128], AF.Sigmoid))
                sc.wait_ge(s_sch, n[0])
                schop(
                    sc.activation(gate_sb[:, 128:160], pg[par][:, 128:160], AF.Tanh)
                )
                sc.wait_ge(s_sch, n[0])
                schop(sc.copy(p5_sb[:, :], pg[par][:, 160:192]))
                sc.wait_ge(s_vch, vidx(t, 3))  # cn ready
                sc.wait_ge(s_sch, n[0])
                schop(
                    sc.activation(
                        chain[:, 3 * 32 : 4 * 32], chain[:, 2 * 32 : 3 * 32], AF.Tanh
                    )
                )
                assert n[0] == M + APS * t + 4

        # ---------------- vector engine (DVE) ---------------------------
        @block.vector
        def _(ve):
            nv = [0]

            def vop(instr):
                nv[0] += 1
                instr.then_inc(s_vch, 1)
                return instr

            def vwait():
                if nv[0]:
                    ve.wait_ge(s_vch, nv[0])

            # phase-2 state init
            ve.wait_ge(s_wdma, 16 * 12)
            vop(ve.memset(hslot[:, 0:128], 0.0))
            vwait()
            vop(ve.memset(c_sb[:, 0:32], 0.0))
            vwait()
            vop(ve.memset(scm[:, :], 0.0))
            t1 = chain[:, 0:32]
            t2 = chain[:, 32:64]
            cn = chain[:, 64:96]
            th = chain[:, 96:128]
            A = chain[:, 128:160]
            Bv = chain[:, 160:192]
            Cv = chain[:, 192:224]
            Ov = chain[:, 224:256]
            for t in range(T):
                par = t % 2
                parn = (t + 1) % 2
                ve.wait_ge(s_sch, sidx(t, 2))
                vwait()
                vop(ve.tensor_mul(t1, gate_sb[:, 0:32], gate_sb[:, 128:160]))
                vwait()
                vop(ve.tensor_mul(t2, gate_sb[:, 32:64], c_sb[:, par * 32 : par * 32 + 32]))
                vwait()
                vop(ve.tensor_add(cn, t1, t2))
                ve.wait_ge(s_sch, sidx(t, 4))
                vwait()
                vop(ve.tensor_mul(A, gate_sb[:, 64:96], th))
                vwait()
                vop(ve.tensor_sub(Bv, p5_sb[:, :], A))
                vwait()
                vop(ve.tensor_mul(Cv, gate_sb[:, 96:128], Bv))
                vwait()
                vop(ve.tensor_add(Ov, A, Cv))
                half = (t // 16) % 2
                if t >= 32 and t % 16 == 0:
                    kk = t // 16
                    ve.wait_ge(s_od[kk % 2], 16 * ((kk - 2) // 2 + 1))
                oof = half * 512 + (t % 16) * 32
                om = outstage[:, oof : oof + 32]
                vwait()
                vop(ve.tensor_scalar_mul(om, Ov, mask_sb[:, t : t + 1]))
                if t >= 2:
                    ve.wait_ge(s_ls[(t + 1) % 2], 16 * ((t - 2) // 2 + 1))
                vwait()
                vop(ve.transpose(scm[:, parn * 32 : parn * 32 + 32], om))
                vwait()
                vop(
                    ve.tensor_scalar_mul(
                        c_sb[:, parn * 32 : parn * 32 + 32], cn, mask_sb[:, t : t + 1]
                    )
                )
                assert nv[0] == VINIT + VPS * t + 10

        # ---------------- gpsimd: exchange ------------------------------
        @block.gpsimd
        def _(g):
            gidx = g.alloc_register("gidx")
            g.reg_load(gidx, soff_d[0:1, 0:1])
            sg = g.snap(gidx, donate=True, min_val=0, max_val=3)
            rd = [None] * 8
            for dd in range(4):
                rd[dd] = (0, dd)

            def prep(t):
                pn = (t + 1) % 2
                g.remote_dma_broadcast(
                    hslot[:, bass.ds(sg * 32 + pn * 128, 32)],
                    scm[:, pn * 32 : pn * 32 + 32],
                    remote_sem=s_rs[pn],
                    local_sem=s_ls[pn],
                    rdests=rd,
                ).then_inc(s_prep, 1)

            prep(0)
            for t in range(T):
                g.wait_ge(s_vch, 3 + 10 * t + 9)
                g.wait_ge(s_prep, t + 1)
                g.trigger_dma(count=1)
                if t + 1 < T:
                    prep(t + 1)
            g.wait_ge(s_rs[0], 8 * (T // 2))
            g.wait_ge(s_rs[1], 8 * (T // 2))
            g.wait_ge(s_ls[0], 16 * (T // 2))
            g.wait_ge(s_ls[1], 16 * (T // 2))

    nc.compile()
    return nc


_BUILD_CACHE = {}


def _get_nc(T):
    if T not in _BUILD_CACHE:
        _BUILD_CACHE[T] = build_nc(T)
    return _BUILD_CACHE[T]


def kernel(x, lengths, w_in, b_in, w_state, b_state):
    from concourse.bass_utils import run_bass_kernel_spmd

    x = np.asarray(x, dtype=np.float32)
    lengths = np.asarray(lengths).astype(np.int64)
    w_in = np.asarray(w_in, dtype=np.float32)
    b_in = np.asarray(b_in, dtype=np.float32)
    w_state = np.asarray(w_state, dtype=np.float32)
    b_state = np.asarray(b_state, dtype=np.float32)
    T = x.shape[1]

    nc = _get_nc(T)
    in_maps = [
        prep_core(r, x, lengths, w_in, b_in, w_state, b_state, T) for r in range(8)
    ]
    res = run_bass_kernel_spmd(nc, in_maps, core_ids=list(range(8)))

    out = np.zeros((B, T, H), dtype=np.float32)
    for r in range(8):
        g, bh = r & 3, r >> 2
        od = np.asarray(res.results[r]["outd"]).astype(np.float32)  # [128, T*32]
        od = od.reshape(128, T, 32)
        for s in range(4):
            blk = od[32 * s : 32 * s + 32]  # [32 bhat, T, 32]
            out[
                32 * bh : 32 * bh + 32, :, 128 * g + 32 * s : 128 * g + 32 * s + 32
            ] = blk
    return out


if __name__ == "__main__":
    import jax

    sys.path.insert(0, os.path.dirname(os.path.abspath(__file__)))
    import reference

    with jax.default_device(jax.devices("cpu")[0]):
        inputs = {k: np.asarray(v) for k, v in reference.setup_inputs().items()}
        expected = np.asarray(reference.reference(**inputs))
    got = kernel(**inputs)
    err = np.abs(got - expected)
    print(
        f"max_abs_err={err.max():.3e} rel={err.max() / np.abs(expected).max():.3e}"
    )
